# revision 1
# baseline (speedup 1.0000x reference)
"""Bahdanau-attention Trainium2 kernel (data-parallel over 8 NeuronCores).

Computation (per batch row b):
    energy[s, d] = tanh(hidden[b] @ W_h + enc[b, s] @ W_e + b_attn)   [S, D]
    scores[s]    = energy[s] . w_v                                     [S]
    attn         = softmax(scores)                                     [S]
    out[b]       = sum_s attn[s] * enc[b, s]                           [E]

Device mapping (per core, 8 batches), v5 — PE-centric:
  - enc staged host-side twice in bf16: encT [b, e, s] (e on partitions) for
    the energy matmul, encN [b, s, e] (s on partitions) for the weighted sum.
  - energy: W_e chunks stationary, encT streams -> psum [d, s]; tanh+bias
    fused on ScalarE (per-partition bias column h_projT[:, b]+b_attn);
    w_v multiply (DVE tensor_scalar, 4x) and pairwise chunk adds (DVE, 2x).
  - score COLUMNS via PE: asum 128-col slices stationary x ones -> [s, 1]
    columns, so no transposes are ever needed.
  - softmax without max-subtraction (scores are O(1)):
    probsT = exp(scoresT) on ScalarE; sum(exp) via a PE ones-matvec; the
    1/sum normalizer is applied on the final output-row copy.
  - weighted sum on PE: probsT columns stationary, encN tiles stream,
    accumulating K=2048 in 16 chunks into a [1, 512] psum row per batch.
  - weighted sums run one group behind compute, interleaved between blocks,
    so PE consumes them as filler with all dependencies long resolved.
"""

import os
import numpy as np

B, S, ENC, DEC = 64, 2048, 512, 512
NCORES = 8
BL = B // NCORES          # batches per core
P = 128
EC = ENC // P             # 4 e-chunks
DC = DEC // P             # 4 d-chunks
KC = DEC // P             # 4 k-chunks (hidden dim)
ST = 512                  # matmul moving free-dim tile
HT = 1024                 # psum energy tile free size
NSC = S // P              # 16 s-chunks for the weighted sum
NW = 2                    # s-chunks packed per encN DMA tile

_PROGRAM = None


def _build_program():
    import concourse.mybir as mybir
    import concourse.tile as tile
    from concourse import bacc
    from contextlib import ExitStack

    fp32 = mybir.dt.float32
    bf16 = mybir.dt.bfloat16
    AF = mybir.ActivationFunctionType
    ALU = mybir.AluOpType

    nc = bacc.Bacc("TRN2", debug=False, target_bir_lowering=False,
                   num_devices=NCORES)

    enc_d = nc.dram_tensor("encT", [BL, EC, P, S], bf16, kind="ExternalInput").ap()
    encn_d = nc.dram_tensor("encN", [BL, S, ENC], bf16, kind="ExternalInput").ap()
    hid_d = nc.dram_tensor("hiddenT", [KC, P, BL], fp32, kind="ExternalInput").ap()
    wh_d = nc.dram_tensor("whT", [KC, P, DEC], fp32, kind="ExternalInput").ap()
    we_d = nc.dram_tensor("weT", [EC, P, DEC], bf16, kind="ExternalInput").ap()
    battn_d = nc.dram_tensor("battn", [P, DC], fp32, kind="ExternalInput").ap()
    wv_d = nc.dram_tensor("wv", [P, DC], fp32, kind="ExternalInput").ap()
    out_d = nc.dram_tensor("out", [BL, ENC], fp32, kind="ExternalOutput").ap()

    with tile.TileContext(nc) as tc, ExitStack() as ctx:
        const = ctx.enter_context(tc.tile_pool(name="const", bufs=1))
        ps_e = ctx.enter_context(tc.tile_pool(name="ps_e", bufs=2, space="PSUM"))
        ps_sc = ctx.enter_context(tc.tile_pool(name="ps_sc", bufs=3, space="PSUM"))
        enc_pool = ctx.enter_context(tc.tile_pool(name="encp", bufs=12))
        encn_pool = ctx.enter_context(tc.tile_pool(name="encnp", bufs=20))
        tanh_pool = ctx.enter_context(tc.tile_pool(name="tanhp", bufs=4))
        wve_pool = ctx.enter_context(tc.tile_pool(name="wvep", bufs=6))
        wvs_pool = ctx.enter_context(tc.tile_pool(name="wvsp", bufs=4))
        sct_pool = ctx.enter_context(tc.tile_pool(name="sctp", bufs=3))
        stage_pool = ctx.enter_context(tc.tile_pool(name="stagep", bufs=3))

        we_sb = const.tile([P, EC, DEC], bf16)
        wh_sb = const.tile([P, KC, DEC], fp32)
        hid_sb = const.tile([P, KC, BL], fp32)
        battn_sb = const.tile([P, DC], fp32)
        wv_sb = const.tile([P, DC], fp32)
        ones_sb = const.tile([P, 1], bf16)
        biasT_sb = const.tile([P, DC, BL], fp32)
        ssum_sb = const.tile([1, BL], fp32)
        rs_sb = const.tile([1, BL], fp32)

        # weights split into per-chunk pieces so the first matmul isn't
        # gated on a ~22GB/s single-queue megaload
        for c in range(EC):
            nc.sync.dma_start(we_sb[:, c, :], we_d[c])
        for c in range(KC):
            nc.scalar.dma_start(wh_sb[:, c, :], wh_d[c])
        nc.scalar.dma_start(hid_sb[:], hid_d.rearrange("c p b -> p c b"))
        nc.scalar.dma_start(battn_sb[:], battn_d)
        nc.scalar.dma_start(wv_sb[:], wv_d)
        nc.vector.memset(ones_sb[:], 1.0)

        # h_projT[d, b] = sum_k W_h[k, d] * hidden[b, k]; biasT = h_projT + b_attn
        for dc in range(DC):
            hp_ps = ps_sc.tile([P, BL], fp32, tag="sc")
            for kc in range(KC):
                nc.tensor.matmul(
                    hp_ps[:],
                    lhsT=wh_sb[:, kc, dc * P:(dc + 1) * P],
                    rhs=hid_sb[:, kc, :],
                    start=(kc == 0), stop=(kc == KC - 1))
            nc.scalar.activation(biasT_sb[:, dc, :], hp_ps[:], AF.Identity,
                                 bias=battn_sb[:, dc:dc + 1])

        enc_t, encn_t = {}, {}
        pending = []          # deferred weighted-sum bursts (closures)
        eager = {}            # last batch's in-flight weighted-sum state

        def wsum_burst(b, probsT_b):
            def run():
                # normalizer 1/sum(exp(s)) for batch b
                sums_ps = ps_sc.tile([1, NSC], fp32, tag="sc",
                                     name=f"sums{b}")
                nc.tensor.matmul(sums_ps[:], lhsT=ones_sb[:],
                                 rhs=probsT_b[:], start=True, stop=True)
                nc.vector.tensor_reduce(ssum_sb[0:1, b:b + 1], sums_ps[:],
                                        axis=mybir.AxisListType.X, op=ALU.add)
                nc.vector.reciprocal(rs_sb[0:1, b:b + 1],
                                     ssum_sb[0:1, b:b + 1])
                orow = ps_sc.tile([1, ENC], fp32, tag="sc", name=f"orow{b}")
                for sc in range(NSC):
                    nc.tensor.matmul(
                        orow[:], lhsT=probsT_b[:, sc:sc + 1],
                        rhs=encn_t[(b, sc // NW)][:, sc % NW, :],
                        start=(sc == 0), stop=(sc == NSC - 1))
                ostg = stage_pool.tile([1, ENC], fp32, tag="stg",
                                       name=f"ostg{b}")
                nc.scalar.activation(ostg[:], orow[:], AF.Copy,
                                     scale=rs_sb[0:1, b:b + 1])
                nc.sync.dma_start(out_d[b:b + 1, :], ostg[:])
                for w in range(NSC // NW):
                    encn_t.pop((b, w))
            return run

        def issue_encn(b):
            # natural-layout tiles for the weighted sum, NW s-chunks per
            # DMA (fewer, bigger issues — the SP issue rate is precious)
            for w in range(NSC // NW):
                t = encn_pool.tile([P, NW, ENC], bf16, tag="encn",
                                   name=f"encn{b}_{w}")
                nc.sync.dma_start(
                    t[:], encn_d[b, w * NW * P:(w + 1) * NW * P, :]
                    .rearrange("(c p) e -> p c e", p=P))
                encn_t[(b, w)] = t

        for b in range(BL):
            if True:
                # split tile loads across DMA queues (one queue moves only
                # ~22GB/s; a single 512KB load would take ~22us); issue the
                # pieces chunk-interleaved so the first accumulation group's
                # inputs all land early
                nsplit = 4 if b == 0 else 2
                w = S // nsplit
                for ec in range(EC):
                    t = enc_pool.tile([P, S], bf16, tag="enc",
                                      name=f"enc{b}_{ec}")
                    for pc in range(nsplit):
                        nc.sync.dma_start(t[:, pc * w:(pc + 1) * w],
                                          enc_d[b, ec, :, pc * w:(pc + 1) * w])
                    enc_t[(b, ec)] = t
                issue_encn(b)

            scT_g = sct_pool.tile([P, NSC], fp32, tag="sct",
                                  name=f"scT{b}")

            for j in [0]:
                for h in range(S // HT):
                    wve = {}
                    for dc in range(DC):
                        eps = ps_e.tile([P, HT], fp32, tag="pse",
                                        name=f"eps{b}_{h}_{dc}")
                        for st in range(HT // ST):
                            for ec in range(EC):
                                nc.tensor.matmul(
                                    eps[:, st * ST:(st + 1) * ST],
                                    lhsT=we_sb[:, ec, dc * P:(dc + 1) * P],
                                    rhs=enc_t[(b, ec)][:, h * HT + st * ST:
                                                       h * HT + (st + 1) * ST],
                                    start=(ec == 0), stop=(ec == EC - 1))
                        tanh_t = tanh_pool.tile([P, HT], bf16, tag="tanh",
                                                name=f"tanh{b}_{h}_{dc}")
                        nc.scalar.activation(tanh_t[:], eps[:], AF.Tanh,
                                             bias=biasT_sb[:, dc, b:b + 1])
                        wve_t = wve_pool.tile([P, HT], bf16, tag="wve",
                                              name=f"wve{b}_{h}_{dc}")
                        nc.vector.tensor_scalar_mul(wve_t[:], tanh_t[:],
                                                    wv_sb[:, dc:dc + 1])
                        wve[dc] = wve_t

                    a01 = wvs_pool.tile([P, HT], bf16, tag="wvs",
                                        name=f"a01_{b}_{h}")
                    nc.vector.tensor_add(a01[:], wve[0][:], wve[1][:])
                    a23 = wvs_pool.tile([P, HT], bf16, tag="wvs",
                                        name=f"a23_{b}_{h}")
                    nc.vector.tensor_add(a23[:], wve[2][:], wve[3][:])
                    asum = wvs_pool.tile([P, HT], bf16, tag="wvs",
                                         name=f"asum{b}_{h}")
                    nc.vector.tensor_add(asum[:], a01[:], a23[:])

                    # score COLUMNS for this half: asum 128-col slices
                    # stationary x ones -> scoresT[s, 1] per chunk
                    nh = HT // P
                    scth = ps_sc.tile([P, nh], fp32, tag="sc",
                                      name=f"scth{b}_{h}")
                    for sci in range(nh):
                        nc.tensor.matmul(scth[:, sci:sci + 1],
                                         lhsT=asum[:, sci * P:(sci + 1) * P],
                                         rhs=ones_sb[:],
                                         start=True, stop=True)
                    nc.vector.tensor_copy(
                        scT_g[:, h * nh:(h + 1) * nh], scth[:])

                    # previous batch's weighted sum interleaves here
                    if (h == 1) and pending:
                        pending.pop(0)()

                # release the batch's encT tiles (only this phase reads them)
                for ec in range(EC):
                    enc_t.pop((b, ec))

            probsT_b = sct_pool.tile([P, NSC], bf16, tag="probst",
                                     name=f"probsT{b}")
            nc.scalar.activation(probsT_b[:], scT_g[:], AF.Exp)
            pending.append(wsum_burst(b, probsT_b))

        while pending:
            pending.pop(0)()

    nc.compile()
    return nc


def _get_program():
    global _PROGRAM
    if _PROGRAM is None:
        _PROGRAM = _build_program()
    return _PROGRAM


def _make_in_maps(hidden, encoder_outputs, W_attn, b_attn, w_v):
    import ml_dtypes
    bf = ml_dtypes.bfloat16
    W_h, W_e = W_attn[:DEC], W_attn[DEC:]
    whT = np.ascontiguousarray(np.asarray(W_h, np.float32).reshape(KC, P, DEC))
    weT = np.ascontiguousarray(np.asarray(W_e).reshape(EC, P, DEC).astype(bf))
    battn = np.ascontiguousarray(np.asarray(b_attn, np.float32).reshape(DC, P).T)
    wv = np.ascontiguousarray(np.asarray(w_v, np.float32).reshape(DC, P).T)
    in_maps = []
    for c in range(NCORES):
        hb = np.asarray(hidden[c * BL:(c + 1) * BL], np.float32)
        eb = np.asarray(encoder_outputs[c * BL:(c + 1) * BL])
        hidT = np.ascontiguousarray(hb.T.reshape(KC, P, BL))
        encT = np.ascontiguousarray(
            eb.transpose(0, 2, 1).reshape(BL, EC, P, S).astype(bf))
        encN = np.ascontiguousarray(eb.astype(bf))
        in_maps.append({"encT": encT, "encN": encN, "hiddenT": hidT,
                        "whT": whT, "weT": weT, "battn": battn, "wv": wv})
    return in_maps


def _install_trace_hooks():
    """The agent image's antenv lacks axon_hooks; recreate it from the
    ctypes NTFF profile shim in trn_agent_boot, and stub the fish-bucket
    artifact upload so the trace path stays local."""
    import sys, types
    if "antenv.axon_hooks" not in sys.modules:
        mod = types.ModuleType("antenv.axon_hooks")
        mod._hook = None
        mod.set_axon_ntff_profile_hook = lambda h: setattr(mod, "_hook", h)
        mod.get_axon_ntff_profile_hook = lambda: mod._hook
        sys.modules["antenv.axon_hooks"] = mod
        import antenv
        antenv.axon_hooks = mod
        try:
            from trn_agent_boot.trn_boot import _ntff_profile_via_ctypes
            mod._hook = _ntff_profile_via_ctypes("/opt/axon/libaxon_pjrt.so")
        except Exception as e:
            print(f"NTFF hook install failed: {e}")
    import concourse.bass_utils as bu
    bu.upload_artifacts = lambda tmpdir: f"local:{tmpdir}"


def run(hidden, encoder_outputs, W_attn, b_attn, w_v, trace=False, tmpdir=None):
    from concourse.bass_utils import run_bass_kernel_spmd
    if trace:
        _install_trace_hooks()
    nc = _get_program()
    in_maps = _make_in_maps(hidden, encoder_outputs, W_attn, b_attn, w_v)
    res = run_bass_kernel_spmd(nc, in_maps, list(range(NCORES)),
                               trace=trace, tmpdir=tmpdir)
    out = np.concatenate([np.asarray(res.results[c]["out"], np.float32)
                          for c in range(NCORES)], axis=0)
    return out, res


def kernel(hidden, encoder_outputs, W_attn, b_attn, w_v):
    out, _ = run(hidden, encoder_outputs, W_attn, b_attn, w_v)
    return out

